# revision 1
# baseline (speedup 1.0000x reference)
"""Multi-head causal attention (B=2, S=2048, H=1024, 16 heads) on 8 TRN2
NeuronCores.

Sharding: core c in 0..7 handles batch b = c // 4 and head group g = c % 4
(heads 4g..4g+3).  Each core computes Q/K/V projections for its 4 heads,
causal attention, and the partial output projection through its column slice
of Wo.  The 4 cores of a batch ReduceScatter(add) their [2048, 1024] partials
so core i of the group ends up with rows 512*i..512*i+512 fully reduced; the
host concatenates the chunks.

Device dataflow (per core, all matmuls in float32r — 4x the fp32 PE rate at
~2e-4 relative error):
  - activations pre-transposed on host to [1024, 2048] (feature-major) since
    the PE contracts over the partition dim
  - QT/KT [256, 2048] via weight-stationary matmuls (k-outer over strip
    halves so each fused fp32-class LDWEIGHTS is shared by 2 strips), bias
    fused in the ACT PSUM->SBUF evacuation; V [2048, 256] natural with a
    fused ones column per head (rowsums fall out of the attnV matmul)
  - scores computed transposed (scoresT[k, q]) so the exp'd tiles feed the
    attention-value matmul directly as the stationary operand, no transposes;
    the two heads of a pair occupy PE row groups 0-1/2-3 and run concurrently
  - attention processed over strip groups (3,0), (2), (1): grouped strips
    share every stationary operand, and the unequal causal lengths stagger
    strip completions so each strip's out-projection + ReduceScatter chunk
    overlaps the remaining compute (the last strip's RS is split in half)
  - causal handled by skipping fully-masked 128x512 blocks and multiplying
    the 4 diagonal-block patterns with precomputed 0/1 masks (DVE)
  - softmax normalization: rowsum row (partition 64 of the attnV PSUM tile)
    -> ACT evac -> PE K=1 outer-product broadcast -> DVE reciprocal approx ->
    multiply during PSUM evacuation, writing each head pair's OT into one
    [128, q] tile; 1/sqrt(64) folded into the exp activation scale
  - out projection contracts per head-pair (K=128) over host-split Wo slices
  - the per-strip ReduceScatter and the bias add + output store run entirely
    on GpSimd (trigger, DMA, add) so no compute engine ever waits on the
    collective; a tiny dummy collective at kernel start warms the CC stream
"""

import sys

for _p in ("/opt/trn_rl_repo", "/root/.axon_site/_ro/trn_rl_repo"):
    if _p not in sys.path:
        sys.path.insert(0, _p)

import numpy as np

import concourse.bass as bass
import concourse.tile as tile
from concourse import bacc
import concourse.mybir as mybir

B = 2
S = 2048
HID = 1024
HEADS_PER_CORE = 4
DH = 64  # head dim
HG = HEADS_PER_CORE * DH  # 256: hidden slice per core
N_CORES = 8
GROUP = 4  # cores per batch (reduction group)

F32 = mybir.dt.float32
F32R = mybir.dt.float32r
AF = mybir.ActivationFunctionType
ALU = mybir.AluOpType

KT = 128  # contraction tile (partitions)
QS = 512  # q strip width
NKT = S // KT  # 16 k-tiles
NQS = S // QS  # 4 q strips
NST = S // KT  # 16 s tiles


def build_nc():
    nc = bacc.Bacc(
        "TRN2", target_bir_lowering=False, debug=False, num_devices=N_CORES
    )

    # per-core inputs (already sharded/transposed by the host)
    xq = nc.dram_tensor("xq", [HID, S], F32, kind="ExternalInput").ap()
    xk = nc.dram_tensor("xk", [HID, S], F32, kind="ExternalInput").ap()
    xv = nc.dram_tensor("xv", [HID, S], F32, kind="ExternalInput").ap()
    wq = nc.dram_tensor("wq", [HID, HG], F32, kind="ExternalInput").ap()
    wk = nc.dram_tensor("wk", [HID, HG], F32, kind="ExternalInput").ap()
    wv = nc.dram_tensor("wv", [HID, HG], F32, kind="ExternalInput").ap()
    w2 = nc.dram_tensor("w2", [2, 2 * DH, HID], F32, kind="ExternalInput").ap()
    bqk = nc.dram_tensor("bqk", [2, 2, 128, 1], F32, kind="ExternalInput").ap()
    bvb = nc.dram_tensor("bvb", [128, HG], F32, kind="ExternalInput").ap()
    bob = nc.dram_tensor("bob", [128, HID], F32, kind="ExternalInput").ap()
    msk = nc.dram_tensor("msk", [128, 4, QS], F32, kind="ExternalInput").ap()

    out_chunk = nc.dram_tensor(
        "out_chunk", [S // GROUP, HID], F32, kind="ExternalOutput"
    ).ap()

    out_part = nc.dram_tensor("out_part", [S, HID], F32)
    rs_out = nc.dram_tensor("rs_out", [S // GROUP, HID], F32)
    cc_warm_in = nc.dram_tensor("cc_warm_in", [4, 128], F32)
    cc_warm_out = nc.dram_tensor("cc_warm_out", [1, 128], F32)

    groups = [[0, 1, 2, 3], [4, 5, 6, 7]]

    with tile.TileContext(nc) as tc:
        with (
            tc.tile_pool(name="wpool", bufs=1) as wpool,
            tc.tile_pool(name="qkv", bufs=1) as qkv,
        ):
            # ---- constants / weights ----
            # load order matters for the start ramp: wq/bq first (the first
            # matmul group needs only those), then wk/wv; phase-A-only
            # constants (w2, masks, bo) ride on the scalar ring
            bq_sb = []
            bk_sb = []
            for m in range(2):
                t = wpool.tile([128, 1], F32, tag=f"bq{m}")
                nc.sync.dma_start(t[:], bqk[0, m])
                bq_sb.append(t)
                t = wpool.tile([128, 1], F32, tag=f"bk{m}")
                nc.scalar.dma_start(t[:], bqk[1, m])
                bk_sb.append(t)
            wq_all = wpool.tile([128, 8, HG], F32R, tag="wq")
            nc.sync.dma_start(
                wq_all[:], wq.rearrange("(a b) c -> b a c", b=128).bitcast(F32R)
            )
            wq_sb = [wq_all[:, k] for k in range(8)]
            wk_all = wpool.tile([128, 8, HG], F32R, tag="wk")
            nc.scalar.dma_start(
                wk_all[:], wk.rearrange("(a b) c -> b a c", b=128).bitcast(F32R)
            )
            wk_sb = [wk_all[:, k] for k in range(8)]
            wv_all = wpool.tile([128, 8, HG], F32R, tag="wv")
            nc.scalar.dma_start(
                wv_all[:], wv.rearrange("(a b) c -> b a c", b=128).bitcast(F32R)
            )
            wv_sb = [wv_all[:, k] for k in range(8)]
            bv_sb = wpool.tile([128, HG], F32, tag="bvb")
            nc.scalar.dma_start(bv_sb[:], bvb[:])
            # tiny dummy collective: warms the CC stream so the first real
            # ReduceScatter doesn't pay stream-startup costs
            zt = wpool.tile([4, 128], F32, tag="zt")
            nc.gpsimd.memset(zt[:], 0.0)
            nc.gpsimd.dma_start(cc_warm_in[:], zt[:])
            nc.gpsimd.collective_compute(
                "ReduceScatter",
                ALU.add,
                replica_groups=groups,
                ins=[cc_warm_in[:]],
                outs=[cc_warm_out[:]],
            )

            # ---- persistent activations ----
            # QT/KT: [dh', s] with heads 2t, 2t+1 in partition halves of tile t
            qt_sb = [qkv.tile([128, S], F32R, tag=f"qt{m}", name=f"qt{m}") for m in range(2)]
            kt_sb = [qkv.tile([128, S], F32R, tag=f"kt{m}", name=f"kt{m}") for m in range(2)]
            # V natural [s, (head, dh+1)] with a ones column per head
            v_sb = [qkv.tile([128, HEADS_PER_CORE, DH + 1], F32R, tag=f"v{st}", name=f"v{st}")
                    for st in range(NST)]
            # normalized attention outputs OT, per (pair, strip): [2*dh, q]
            # (head 2p in partitions 0-63, head 2p+1 in partitions 64-127)
            ot_sb = [[qkv.tile([2 * DH, QS], F32R, tag=f"ot{p}_{s4}", name=f"ot{p}_{s4}")
                      for s4 in range(NQS)] for p in range(2)]

            # ---- phase P: projections ----
            # processed in strip-halves (1024 cols): k-outer QK loops share
            # each weight-slice stationary across 2 strips (fused fp32-class
            # LDWEIGHTS can't be pulled ahead, so reuse is the only way to
            # amortize it), and x loads are 512 KB for DMA efficiency
            with tc.tile_pool(name="xs", bufs=4) as xs, \
                 tc.tile_pool(name="xsv", bufs=8) as xsv, \
                 tc.tile_pool(name="pj", bufs=4, space="PSUM") as pj, \
                 tc.tile_pool(name="pv", bufs=4, space="PSUM") as pv:
                for half in range(2):
                    strips = (2 * half, 2 * half + 1)
                    hsl = slice(2 * QS * half, 2 * QS * half + 2 * QS)
                    for w_sb, xdram, sbuf, b_sb, xtag in (
                        (wq_sb, xq, qt_sb, bq_sb, "xq"),
                        (wk_sb, xk, kt_sb, bk_sb, "xk"),
                    ):
                        psq = {
                            (ti, m): pj.tile(
                                [128, QS], F32, tag="pj", name=f"psq{ti}{m}"
                            )
                            for ti in range(2)
                            for m in range(2)
                        }
                        for k in range(8):
                            xt_ = xs.tile([128, 2 * QS], F32R, tag=xtag, name="xt")
                            dma_eng = nc.sync if k % 2 == 0 else nc.scalar
                            dma_eng.dma_start(
                                xt_[:],
                                xdram[128 * k : 128 * k + 128, hsl].bitcast(F32R),
                            )
                            for m in range(2):
                                for ti in range(2):
                                    nc.tensor.matmul(
                                        psq[(ti, m)][:],
                                        w_sb[k][:, 128 * m : 128 * m + 128],
                                        xt_[:, QS * ti : QS * ti + QS],
                                        start=(k == 0),
                                        stop=(k == 7),
                                    )
                        for ti in range(2):
                            sl = slice(QS * strips[ti], QS * strips[ti] + QS)
                            for m in range(2):
                                nc.scalar.activation(
                                    sbuf[m][:, sl], psq[(ti, m)][:], AF.Identity,
                                    bias=b_sb[m][:],
                                )
                    # V s-tiles (x-stationary); k-tiles streamed, 4 psum groups
                    for t in strips:
                        sl = slice(QS * t, QS * t + QS)
                        psv = [pv.tile([128, HG], F32, tag="pv", name=f"psv{u}")
                               for u in range(4)]
                        for k in range(8):
                            xt_ = xsv.tile([128, QS], F32R, tag="xv", name="xt")
                            nc.gpsimd.dma_start(
                                xt_[:],
                                xv[128 * k : 128 * k + 128, sl].bitcast(F32R),
                            )
                            for u in range(4):
                                nc.tensor.matmul(
                                    psv[u][:],
                                    xt_[:, 128 * u : 128 * u + 128],
                                    wv_sb[k],
                                    start=(k == 0),
                                    stop=(k == 7),
                                )
                        for u in range(4):
                            st = 4 * t + u
                            for h in range(HEADS_PER_CORE):
                                nc.vector.tensor_tensor(
                                    v_sb[st][:, h, 0:DH],
                                    psv[u][:, DH * h : DH * h + DH],
                                    bv_sb[:, DH * h : DH * h + DH],
                                    ALU.add,
                                )
                            nc.scalar.activation(
                                v_sb[st][:, :, DH],
                                bv_sb[:, 0:HEADS_PER_CORE],
                                AF.Identity,
                                scale=0.0,
                                bias=1.0,
                            )

            bo_sb = wpool.tile([128, HID], F32, tag="bob")
            nc.sync.dma_start(bo_sb[:], bob[:])
            w2_all = wpool.tile([2 * DH, 2, HID], F32R, tag="w2")
            nc.sync.dma_start(
                w2_all[:], w2.rearrange("a b c -> b a c").bitcast(F32R)
            )
            w2_sb = [w2_all[:, p] for p in range(2)]
            mask_sb = wpool.tile([128, 4, QS], F32R, tag="msk")
            nc.sync.dma_start(mask_sb[:], msk.bitcast(F32R))
            # [1, 64] of ones: stationary operand of the rowsum-broadcast
            # outer-product matmul
            ones_sb = wpool.tile([1, DH], F32R, tag="ones")
            nc.scalar.activation(
                ones_sb[:], bo_sb[0:1, 0:DH], AF.Identity, scale=0.0, bias=1.0
            )

            # ---- phases A+O: attention, out-projection, chunked RS ----
            # s4-outer so each q-strip's output rows complete early and the
            # ReduceScatter chunks overlap with later strips' compute.
            with tc.tile_pool(name="attn", bufs=9) as attn_pool, \
                 tc.tile_pool(name="norm", bufs=4) as norm_pool, \
                 tc.tile_pool(name="osb", bufs=4) as osb_pool, \
                 tc.tile_pool(name="pp", bufs=4, space="PSUM") as psp, \
                 tc.tile_pool(name="po", bufs=4, space="PSUM") as pop:
                pup = psp  # scores, out-proj and rowsum-bcast share slots
                def emit_outproj(s4):
                    # out-projection + reduce-scatter for strip s4 (emitted
                    # lazily, interleaved into the next strip's scores so the
                    # PE never idles on the normalize chain)
                    for u in range(4):
                        st = 4 * s4 + u
                        o = u * 128
                        for eh in range(2):
                            esl = slice(QS * eh, QS * eh + QS)
                            ps = pup.tile([128, QS], F32, tag="pp", name="psu")
                            for p in range(2):
                                nc.tensor.matmul(
                                    ps[:],
                                    ot_sb[p][s4][:, o : o + 128],
                                    w2_sb[p][:, esl],
                                    start=(p == 0),
                                    stop=(p == 1),
                                )
                            osb = osb_pool.tile([128, QS], F32, tag="osb", name="osb")
                            nc.vector.tensor_copy(osb[:], ps[:])
                            nc.sync.dma_start(
                                out_part[128 * st : 128 * st + 128, esl], osb[:]
                            )
                        # reduce-scatter finished output rows; the last strip
                        # is split in half so the final collective is smaller.
                        # Core with group rank r receives the chunk's r-th
                        # quarter; its out_chunk row offset is r0 // 4.
                        if s4 != 1:
                            chunks = [(QS * s4, QS)] if u == 3 else []
                        else:
                            chunks = (
                                [(QS * s4, QS // 2)] if u == 1
                                else [(QS * s4 + QS // 2, QS // 2)] if u == 3
                                else []
                            )
                        for r0, rn in chunks:
                            q = rn // 4
                            nc.gpsimd.collective_compute(
                                "ReduceScatter",
                                ALU.add,
                                replica_groups=groups,
                                ins=[out_part[r0 : r0 + rn].opt()],
                                outs=[rs_out[r0 // 4 : r0 // 4 + q].opt()],
                            )
                            # post-RS path entirely on GpSimd so no compute
                            # engine (PE/ACT/DVE) ever waits on the collective
                            t_in = osb_pool.tile(
                                [128, HID], F32, tag="rs_in", name="rs_in"
                            )
                            nc.gpsimd.dma_start(
                                t_in[0:q], rs_out[r0 // 4 : r0 // 4 + q]
                            )
                            t_out = osb_pool.tile(
                                [128, HID], F32, tag="rs_bias", name="rs_b"
                            )
                            nc.gpsimd.tensor_tensor(
                                t_out[0:q], t_in[0:q], bo_sb[0:q], ALU.add
                            )
                            nc.gpsimd.dma_start(
                                out_chunk[r0 // 4 : r0 // 4 + q], t_out[0:q]
                            )

                # attention over strip GROUPS (1,2) then (0,3): the strips of
                # a group share every stationary operand (K- and V-slices) so
                # the serialized fp32r LDWEIGHTS amortizes, and the unequal
                # causal lengths stagger strip completions so each strip's
                # out-projection + ReduceScatter overlaps remaining compute
                def normalize(pair, s4, pso_t, hh):
                    rs = norm_pool.tile([1, QS], F32R, tag="rs", name="rs")
                    nc.scalar.activation(rs[:], pso_t[DH : DH + 1], AF.Copy)
                    # broadcast rowsums to 64 partitions via a K=1
                    # outer-product matmul on the PE
                    rbc = pup.tile([64, QS], F32, tag="pp", name="rbc")
                    nc.tensor.matmul(
                        rbc[:], ones_sb[:], rs[:], start=True, stop=True
                    )
                    rrec = norm_pool.tile([64, QS], F32, tag="rrec", name="rrec")
                    nc.vector.reciprocal_approx_fast(rrec[:], rbc[:])
                    nc.vector.tensor_tensor(
                        ot_sb[pair][s4][64 * hh : 64 * hh + 64],
                        pso_t[0:DH],
                        rrec[:],
                        ALU.mult,
                    )

                for strips in ((3, 0), (2, 2), (1, 1)):
                    singleton = strips[0] == strips[1]
                    tis = [0] if singleton else [0, 1]
                    nkts = {ti: 4 * strips[ti] + 4 for ti in tis}
                    nktmax = max(nkts.values())
                    for pair in range(2):
                        pso = {
                            (ti, hh): pop.tile(
                                [DH + 1, QS], F32, tag="po", name=f"pso{ti}{hh}"
                            )
                            for ti in tis
                            for hh in range(2)
                        }
                        ats = {}

                        def valid(j):
                            return [ti for ti in tis if j < nkts[ti]]

                        def do_scores(j):
                            ats[j] = {}
                            for hh in range(2):
                                hp = 64 * hh
                                for ti in valid(j):
                                    s4 = strips[ti]
                                    qsl = slice(QS * s4, QS * s4 + QS)
                                    pss = psp.tile(
                                        [128, QS], F32, tag="pp", name="pss"
                                    )
                                    nc.tensor.matmul(
                                        pss[:],
                                        kt_sb[pair][
                                            hp : hp + 64, 128 * j : 128 * j + 128
                                        ],
                                        qt_sb[pair][hp : hp + 64, qsl],
                                        start=True,
                                        stop=True,
                                    )
                                    at = attn_pool.tile(
                                        [128, QS], F32R, tag="at", name="at"
                                    )
                                    nc.scalar.activation(
                                        at[:], pss[:], AF.Exp, scale=1.0 / 8.0
                                    )
                                    if j >= 4 * s4:
                                        i = j - 4 * s4
                                        nc.vector.tensor_tensor(
                                            at[:], at[:], mask_sb[:, i, :], ALU.mult
                                        )
                                    ats[j][(ti, hh)] = at

                        # scores pipelined one k-tile ahead of attnV
                        do_scores(0)
                        ready_outproj = []
                        for j in range(nktmax):
                            if j + 1 < nktmax:
                                do_scores(j + 1)
                            # out-proj for strips whose normalize was emitted
                            # a k-tile ago (the DVE chain has drained by now)
                            for s4r in ready_outproj:
                                emit_outproj(s4r)
                            ready_outproj = []
                            for hh in range(2):
                                h = 2 * pair + hh
                                for ti in valid(j):
                                    nc.tensor.matmul(
                                        pso[(ti, hh)][:],
                                        v_sb[j][:, h, :],
                                        ats[j][(ti, hh)][:],
                                        start=(j == 0),
                                        stop=(j == nkts[ti] - 1),
                                    )
                            del ats[j]
                            for ti in tis:
                                if j == nkts[ti] - 1:
                                    s4 = strips[ti]
                                    for hh in range(2):
                                        normalize(pair, s4, pso[(ti, hh)], hh)
                                    if pair == 1 and j < nktmax - 1:
                                        ready_outproj.append(s4)
                        for s4r in ready_outproj:
                            emit_outproj(s4r)
                    # the group's longest strip drains at the group end
                    emit_outproj(max(strips) if not singleton else strips[0])

    nc.compile()
    return nc


_NC = None
_RUNNER = None


def _get_runner():
    """Build the compiled 8-core PJRT callable once and cache it."""
    global _NC, _RUNNER
    if _RUNNER is not None:
        return _RUNNER

    import jax
    import numpy as _np
    from jax.sharding import Mesh, PartitionSpec
    from jax.experimental.shard_map import shard_map
    from concourse.bass2jax import (
        _bass_exec_p,
        install_neuronx_cc_hook,
        partition_id_tensor,
    )

    _NC = build_nc()
    nc = _NC
    install_neuronx_cc_hook()

    partition_name = nc.partition_id_tensor.name if nc.partition_id_tensor else None
    in_names = []
    out_names = []
    out_avals = []
    zero_outs = []
    for alloc in nc.m.functions[0].allocations:
        if not isinstance(alloc, mybir.MemoryLocationSet):
            continue
        name = alloc.memorylocations[0].name
        if alloc.kind == "ExternalInput":
            if name != partition_name:
                in_names.append(name)
        elif alloc.kind == "ExternalOutput":
            shape = tuple(alloc.tensor_shape)
            dtype = mybir.dt.np(alloc.dtype)
            out_names.append(name)
            out_avals.append(jax.core.ShapedArray(shape, dtype))
            zero_outs.append(_np.zeros(shape, dtype))
    n_params = len(in_names)
    n_outs = len(out_avals)
    all_in_names = list(in_names) + list(out_names)
    if partition_name is not None:
        all_in_names.append(partition_name)
    donate = tuple(range(n_params, n_params + n_outs))

    def _body(*args):
        operands = list(args)
        if partition_name is not None:
            operands.append(partition_id_tensor())
        outs = _bass_exec_p.bind(
            *operands,
            out_avals=tuple(out_avals),
            in_names=tuple(all_in_names),
            out_names=tuple(out_names),
            lowering_input_output_aliases=(),
            sim_require_finite=True,
            sim_require_nnan=True,
            nc=nc,
        )
        return tuple(outs)

    devices = jax.devices()[:N_CORES]
    mesh = Mesh(np.asarray(devices), ("core",))
    in_specs = (PartitionSpec("core"),) * (n_params + n_outs)
    out_specs = (PartitionSpec("core"),) * n_outs
    sharded = jax.jit(
        shard_map(
            _body, mesh=mesh, in_specs=in_specs, out_specs=out_specs, check_rep=False
        ),
        keep_unused=True,
    )

    def run(in_maps):
        per_core = [[_np.asarray(m[name]) for name in in_names] for m in in_maps]
        concat_in = [
            _np.concatenate([per_core[c][i] for c in range(N_CORES)], axis=0)
            for i in range(n_params)
        ]
        concat_zeros = [
            _np.zeros((N_CORES * z.shape[0], *z.shape[1:]), z.dtype)
            for z in zero_outs
        ]
        out_arrs = sharded(*concat_in, *concat_zeros)
        return [
            {
                name: _np.asarray(out_arrs[i]).reshape(
                    N_CORES, *out_avals[i].shape
                )[c]
                for i, name in enumerate(out_names)
            }
            for c in range(N_CORES)
        ]

    _RUNNER = run
    return run


def make_in_maps(query, key, value, Wq, bq, Wk, bk, Wv, bv, Wo, bo):
    query = np.asarray(query, dtype=np.float32)
    key = np.asarray(key, dtype=np.float32)
    value = np.asarray(value, dtype=np.float32)
    Wq = np.asarray(Wq, dtype=np.float32)
    bq = np.asarray(bq, dtype=np.float32)
    Wk = np.asarray(Wk, dtype=np.float32)
    bk = np.asarray(bk, dtype=np.float32)
    Wv = np.asarray(Wv, dtype=np.float32)
    bv = np.asarray(bv, dtype=np.float32)
    Wo = np.asarray(Wo, dtype=np.float32)
    bo = np.asarray(bo, dtype=np.float32)

    xqT = [np.ascontiguousarray(query[b].T) for b in range(B)]
    xkT = [np.ascontiguousarray(key[b].T) for b in range(B)]
    xvT = [np.ascontiguousarray(value[b].T) for b in range(B)]

    # diagonal-block causal masks: mask[k, i, q] = 1 if q >= k + 128*i
    k_idx = np.arange(128)[:, None, None]
    i_idx = np.arange(4)[None, :, None]
    q_idx = np.arange(QS)[None, None, :]
    masks = (q_idx >= k_idx + 128 * i_idx).astype(np.float32)

    bo_b = np.ascontiguousarray(np.broadcast_to(bo, (128, HID)))

    in_maps = []
    for c in range(N_CORES):
        b = c // GROUP
        g = c % GROUP
        hsl = slice(HG * g, HG * g + HG)
        wq_g = np.ascontiguousarray(Wq[hsl].T)  # [1024, 256]
        wk_g = np.ascontiguousarray(Wk[hsl].T)
        wv_g = np.ascontiguousarray(Wv[hsl].T)
        # w2[h] = Wo[:, g*256 + 64h : +64].T  -> [64, 1024]
        w2_g = np.ascontiguousarray(Wo[:, hsl].T.reshape(2, 2 * DH, HID))
        bqk_g = np.stack(
            [bq[hsl].reshape(2, 128), bk[hsl].reshape(2, 128)]
        )  # [2, 2, 128]
        bv_b = np.ascontiguousarray(np.broadcast_to(bv[hsl], (128, HG)))
        in_maps.append(
            {
                "xq": xqT[b],
                "xk": xkT[b],
                "xv": xvT[b],
                "wq": wq_g,
                "wk": wk_g,
                "wv": wv_g,
                "w2": w2_g,
                "bqk": bqk_g,
                "bvb": bv_b,
                "bob": bo_b,
                "msk": masks,
            }
        )
    return in_maps


RS_CHUNKS = [(0, 512), (512, 256), (768, 256), (1024, 512), (1536, 512)]


def assemble_output(results):
    # for RS chunk (r0, rn), core with group rank r holds global rows
    # [r0 + (rn//4)*r, +rn//4) at out_chunk rows [r0//4, +rn//4)
    out = np.empty((B, S, HID), dtype=np.float32)
    for b in range(B):
        for r in range(GROUP):
            chunk = results[GROUP * b + r]["out_chunk"]
            for r0, rn in RS_CHUNKS:
                q = rn // 4
                out[b, r0 + q * r : r0 + q * (r + 1)] = chunk[
                    r0 // 4 : r0 // 4 + q
                ]
    return out


def kernel(**inputs) -> np.ndarray:
    in_maps = make_in_maps(**inputs)
    run = _get_runner()
    results = run(in_maps)
    return assemble_output(results)


if __name__ == "__main__":
    import reference

    inputs = {k: np.asarray(v) for k, v in reference.setup_inputs().items()}
    got = kernel(**inputs)
    want = np.asarray(reference.reference(**inputs))
    err = np.linalg.norm(got - want) / np.linalg.norm(want)
    print("Relative error:", err)



# revision 23
# speedup vs baseline: 1.5191x; 1.5191x over previous
"""Multi-head causal attention (B=2, S=2048, H=1024, 16 heads) on 8 TRN2
NeuronCores — v2 (bf16).

Sharding: core c in 0..7 handles batch b = c // 4 and head group g = c % 4
(heads 4g..4g+3).  Each core computes Q/K/V projections for its 4 heads and
causal attention.  The out-projection is distributed by sequence: after each
q-strip's attention, the 4 cores of a batch AllToAll their [256-feature,
512-q] attention outputs so core r holds all 1024 features for its 128-row
q-quarter, then projects through the full Wo locally.  The host concatenates
the row blocks.

Key differences vs v1 (fp32r + ReduceScatter, 431us):
  - everything bf16: half the DMA bytes, 1 cycle/row matmuls with separable
    (pull-ahead) LDWEIGHTS instead of the fused fp32-class weight loads
  - per-strip pipeline: projections of strip s+1 interleave with attention
    of strip s (causal attention for strip s only needs K/V strips <= s),
    keeping the PE dense so the HAM clock gate stays at 8/8
  - bias algebra: K bias dropped entirely (softmax-invariant), V bias folded
    into the output bias on the host (softmax rows sum to 1), Q bias applied
    by DVE during PSUM evacuation
  - exp batched per (pair, j) across both head-halves: one [128, 1024] ACT
    instruction over 2 PSUM banks; ACT does nothing else
  - causal masking: only the diagonal 128x128 triangle is multiplied (DVE);
    fully-masked columns of diagonal tiles are skipped in the attnV matmul
  - collective: per-strip bf16 ReduceScatter (1MB in, vs v1's 2MB fp32) of
    the partial out-projection, overlapped with the next strip's attention;
    only the last strip's collective is exposed
"""

import sys

for _p in ("/opt/trn_rl_repo", "/root/.axon_site/_ro/trn_rl_repo"):
    if _p not in sys.path:
        sys.path.insert(0, _p)

import numpy as np

import concourse.bass as bass
import concourse.tile as tile
from concourse import bacc
import concourse.mybir as mybir

B = 2
S = 2048
HID = 1024
HPC = 4  # heads per core
DH = 64  # head dim
HG = HPC * DH  # 256: hidden slice per core
N_CORES = 8
GROUP = 4  # cores per batch (collective group)

F32 = mybir.dt.float32
F32R = mybir.dt.float32r
BF = mybir.dt.bfloat16
AF = mybir.ActivationFunctionType
ALU = mybir.AluOpType

KT = 128  # k tile (contraction positions per tile)
QS = 512  # q strip width
NQS = S // QS  # 4 q strips
NST = S // KT  # 16 k tiles


def build_nc():
    nc = bacc.Bacc(
        "TRN2", target_bir_lowering=False, debug=False, num_devices=N_CORES
    )

    # per-core inputs (sharded/transposed/bf16-cast by the host)
    xq = nc.dram_tensor("xq", [HID, S], BF, kind="ExternalInput").ap()
    xk = nc.dram_tensor("xk", [HID, S], BF, kind="ExternalInput").ap()
    xv = nc.dram_tensor("xv", [HID, S], BF, kind="ExternalInput").ap()
    wq = nc.dram_tensor("wq", [HID, HG], BF, kind="ExternalInput").ap()
    wk = nc.dram_tensor("wk", [HID, HG], BF, kind="ExternalInput").ap()
    wv = nc.dram_tensor("wv", [HID, HG], BF, kind="ExternalInput").ap()
    w2 = nc.dram_tensor("w2", [HG, HID], BF, kind="ExternalInput").ap()
    bqv = nc.dram_tensor("bqv", [128, 2], F32, kind="ExternalInput").ap()
    bob = nc.dram_tensor("bob", [128, HID], F32, kind="ExternalInput").ap()
    trim = nc.dram_tensor("trim", [128, 128], BF, kind="ExternalInput").ap()

    out_chunk = nc.dram_tensor(
        "out_chunk", [NQS, 128, HID], F32, kind="ExternalOutput"
    ).ap()

    out_part = nc.dram_tensor("out_part", [NQS, QS, HID], BF)
    rs_out = nc.dram_tensor("rs_out", [NQS, 128, HID], BF)
    warm_in = nc.dram_tensor("warm_in", [GROUP, 128], F32)
    warm_out = nc.dram_tensor("warm_out", [1, 128], F32)

    groups = [[0, 1, 2, 3], [4, 5, 6, 7]]

    with tile.TileContext(nc) as tc:
        with (
            tc.tile_pool(name="wpool", bufs=1) as wpool,
            tc.tile_pool(name="qkv", bufs=1) as qkv,
            tc.tile_pool(name="xs", bufs=2) as xs,
            tc.tile_pool(name="atp", bufs=5) as atp,
            tc.tile_pool(name="otp", bufs=2) as otp,
            tc.tile_pool(name="osb", bufs=2) as osbp,
            tc.tile_pool(name="nrm", bufs=3) as nrm,
            tc.tile_pool(name="pbig", bufs=2, space="PSUM") as pbig,
            tc.tile_pool(name="ppso", bufs=2, space="PSUM") as ppso,
            tc.tile_pool(name="psml", bufs=2, space="PSUM") as psml,
        ):
            # ---- weights / constants ----
            bq_sb = wpool.tile([128, 2], F32, tag="bq")
            nc.sync.dma_start(bq_sb[:], bqv[:])
            wq_all = wpool.tile([128, 8, HG], BF, tag="wq")
            nc.sync.dma_start(
                wq_all[:], wq.rearrange("(a b) c -> b a c", b=128)
            )
            wk_all = wpool.tile([128, 8, HG], BF, tag="wk")
            nc.scalar.dma_start(
                wk_all[:], wk.rearrange("(a b) c -> b a c", b=128)
            )
            wv_all = wpool.tile([128, 8, HG], BF, tag="wv")
            nc.gpsimd.dma_start(
                wv_all[:], wv.rearrange("(a b) c -> b a c", b=128)
            )
            tri_sb = wpool.tile([128, 128], BF, tag="tri")
            nc.sync.dma_start(tri_sb[:], trim[:])
            bob_sb = wpool.tile([128, HID], F32, tag="bob")
            nc.sync.dma_start(bob_sb[:], bob[:])
            # ones row for the rowsum-broadcast outer-product matmul
            ones_sb = wpool.tile([1, DH], BF, tag="ones")
            nc.vector.memset(ones_sb[:], 1.0)
            # w2 (own column slice of Wo, transposed) rides later on the
            # scalar queue; first needed at out-projection of strip 0
            w2_all = wpool.tile([128, 2, HID], BF, tag="w2")
            nc.scalar.dma_start(
                w2_all[:], w2.rearrange("(t f) e -> f t e", f=128)
            )

            # warm the CC stream so the first real ReduceScatter is cheap
            zt = wpool.tile([GROUP, 128], F32, tag="zt")
            nc.gpsimd.memset(zt[:], 0.0)
            nc.gpsimd.dma_start(warm_in[:], zt[:])
            nc.gpsimd.collective_compute(
                "ReduceScatter",
                ALU.add,
                replica_groups=groups,
                ins=[warm_in[:]],
                outs=[warm_out[:]],
            )

            # ---- persistent activations ----
            # QT/KT per (pair, strip): [dh', q] with heads 2p, 2p+1 in
            # partition halves
            qt_sb = [
                [
                    qkv.tile([128, QS], BF, tag=f"qt{p}{s}", name=f"qt{p}{s}")
                    for s in range(NQS)
                ]
                for p in range(2)
            ]
            kt_sb = [
                [
                    qkv.tile([128, QS], BF, tag=f"kt{p}{s}", name=f"kt{p}{s}")
                    for s in range(NQS)
                ]
                for p in range(2)
            ]
            # V natural [k, (head, dh+1)]: col DH of each head block is the
            # ones column (rowsums fall out of the attnV matmul, row DH)
            v_sb = [
                qkv.tile([128, HPC, DH + 1], BF, tag=f"v{st}", name=f"v{st}")
                for st in range(NST)
            ]
            for st in range(NST):
                nc.vector.memset(v_sb[st][:, :, DH : DH + 1], 1.0)

            # ---- projection steps for one strip (emitted lazily) ----
            def proj_steps(s):
                """Return a list of closures; each emits a small chunk of the
                strip-s projection work so it can interleave with attention
                of strip s-1."""
                steps = []
                sl = slice(QS * s, QS * s + QS)

                xt = {}

                def load_x():
                    for dram, tag, eng in (
                        (xq, "xq", nc.sync),
                        (xk, "xk", nc.scalar),
                        (xv, "xv", nc.gpsimd),
                    ):
                        t = xs.tile([128, 8, QS], BF, tag=tag, name=f"x{tag}")
                        eng.dma_start(
                            t[:],
                            dram.rearrange("(a b) c -> b a c", b=128)[:, :, sl],
                        )
                        xt[tag] = t

                steps.append(load_x)

                # Q then K: k-contiguous accumulation into 2 m-tiles
                psq = {}

                def qk_mm(tag, w_all, k):
                    def f():
                        if k == 0:
                            psq[0] = psml.tile(
                                [128, QS], F32, tag="sml", name=f"ps{tag}0"
                            )
                            psq[1] = psml.tile(
                                [128, QS], F32, tag="sml", name=f"ps{tag}1"
                            )
                        for m in range(2):
                            nc.tensor.matmul(
                                psq[m][:],
                                w_all[:, k, 128 * m : 128 * m + 128],
                                xt[tag][:, k, :],
                                start=(k == 0),
                                stop=(k == 7),
                            )

                    return f

                def q_evac():
                    for m in range(2):
                        nc.vector.tensor_scalar_add(
                            qt_sb[m][s][:], psq[m][:], bq_sb[:, m : m + 1]
                        )

                def k_evac():
                    for m in range(2):
                        nc.vector.tensor_copy(kt_sb[m][s][:], psq[m][:])

                for k in range(8):
                    steps.append(qk_mm("xq", wq_all, k))
                steps.append(q_evac)
                for k in range(8):
                    steps.append(qk_mm("xk", wk_all, k))
                steps.append(k_evac)

                # V: 4 sequential 128-row sub-tiles, x-stationary
                psv = {}

                def v_mm(u, k2):
                    def f():
                        if k2 == 0:
                            psv[u] = psml.tile(
                                [128, QS], F32, tag="sml", name=f"psv{u}"
                            )
                        for k in (2 * k2, 2 * k2 + 1):
                            nc.tensor.matmul(
                                psv[u][:, 0:HG],
                                xt["xv"][:, k, 128 * u : 128 * u + 128],
                                wv_all[:, k, :],
                                start=(k == 0),
                                stop=(k == 7),
                            )

                    return f

                def v_evac(u):
                    def f():
                        st = 4 * s + u
                        nc.vector.tensor_copy(
                            v_sb[st][:, :, 0:DH],
                            psv[u][:, 0:HG].rearrange(
                                "p (h d) -> p h d", h=HPC
                            ),
                        )
                        del psv[u]

                    return f

                for u in range(4):
                    for k2 in range(4):
                        steps.append(v_mm(u, k2))
                    steps.append(v_evac(u))
                return steps

            # ---- out-projection + ReduceScatter for one strip ----
            # partial out rows [512, 1024] from this core's 256 features ->
            # DRAM bf16 -> RS(add) over the 4-core group -> own 128-row
            # quarter.  The ot tiles are captured by reference via `ots`.
            def outproj_steps(s, ots):
                steps = []
                po = {}

                def mm(u, eh):
                    def f():
                        if eh == 0:
                            po[0] = psml.tile(
                                [128, QS], F32, tag="sml", name="po0"
                            )
                            po[1] = psml.tile(
                                [128, QS], F32, tag="sml", name="po1"
                            )
                        for p in range(2):
                            nc.tensor.matmul(
                                po[eh][:],
                                ots[p][:, 128 * u : 128 * u + 128],
                                w2_all[:, p, QS * eh : QS * eh + QS],
                                start=(p == 0),
                                stop=(p == 1),
                            )

                    return f

                def evac(u):
                    def f():
                        ob = osbp.tile([128, 2 * QS], BF, tag="osb", name="osb")
                        for eh in range(2):
                            nc.vector.tensor_copy(
                                ob[:, QS * eh : QS * eh + QS], po[eh][:]
                            )
                        nc.sync.dma_start(
                            out_part[s, 128 * u : 128 * u + 128], ob[:]
                        )

                    return f

                for u in range(4):
                    steps.append(mm(u, 0))
                    steps.append(mm(u, 1))
                    steps.append(evac(u))

                def rs_trigger():
                    nc.gpsimd.collective_compute(
                        "ReduceScatter",
                        ALU.add,
                        replica_groups=groups,
                        ins=[out_part[s]],
                        outs=[rs_out[s]],
                    )

                steps.append(rs_trigger)
                return steps

            # post-RS: load own quarter, add bias, store fp32 (emitted late
            # so the waiting instructions don't clog engine queues)
            def post_steps(s):
                def f():
                    t_in = osbp.tile([128, HID], BF, tag="rsin", name="rsin")
                    nc.scalar.dma_start(t_in[:], rs_out[s])
                    ob = osbp.tile([128, HID], F32, tag="rsout", name="rsout")
                    nc.vector.tensor_tensor(ob[:], t_in[:], bob_sb[:], ALU.add)
                    nc.scalar.dma_start(out_chunk[s], ob[:])

                return [f]

            # ---- main pipeline ----
            pending = proj_steps(0)
            while pending:
                pending.pop(0)()

            pending = []
            post_pending = []

            def pump(n):
                for _ in range(min(n, len(pending))):
                    pending.pop(0)()

            for s in range(NQS):
                if s + 1 < NQS:
                    pending += proj_steps(s + 1)
                jmax = 4 * s + 4
                # interleave budget: spread pending steps over this strip's
                # (pair, j) iterations
                iters = 2 * jmax
                rate = (len(pending) + 2 + iters - 1) // iters

                ot = [
                    otp.tile([128, QS], BF, tag=f"ot{p}", name=f"ot{p}")
                    for p in range(2)
                ]
                for p in range(2):
                    pso = {}
                    ats = {}

                    def do_scores(j):
                        psc = pbig.tile(
                            [128, 2 * QS], F32, tag="big", name="psc"
                        )
                        for hh in range(2):
                            hp = 64 * hh
                            nc.tensor.matmul(
                                psc[:, QS * hh : QS * hh + QS],
                                kt_sb[p][j // 4][
                                    hp : hp + 64, 128 * (j % 4) : 128 * (j % 4) + 128
                                ],
                                qt_sb[p][s][hp : hp + 64, :],
                                start=True,
                                stop=True,
                            )
                        at = atp.tile([128, 2 * QS], BF, tag="at", name="at")
                        nc.scalar.activation(
                            at[:], psc[:], AF.Exp, scale=1.0 / 8.0
                        )
                        if j >= 4 * s:
                            i = j - 4 * s
                            for hh in range(2):
                                o = QS * hh + 128 * i
                                nc.vector.tensor_tensor(
                                    at[:, o : o + 128],
                                    at[:, o : o + 128],
                                    tri_sb[:],
                                    ALU.mult,
                                )
                        ats[j] = at

                    do_scores(0)
                    for j in range(jmax):
                        if j + 1 < jmax:
                            do_scores(j + 1)
                        pump(rate)
                        off = 128 * (j - 4 * s) if j >= 4 * s else 0
                        for hh in range(2):
                            if j == 0:
                                pso[hh] = ppso.tile(
                                    [DH + 1, QS], F32, tag="pso", name=f"pso{hh}"
                                )
                            nc.tensor.matmul(
                                pso[hh][:, off:QS],
                                v_sb[j][:, 2 * p + hh, :],
                                ats[j][:, QS * hh + off : QS * hh + QS],
                                start=(j == 0),
                                stop=(j == jmax - 1),
                            )
                        del ats[j]
                    # normalize: rowsum (psum row DH) -> reciprocal ->
                    # multiply into the persistent OT tile
                    for hh in range(2):
                        rs = nrm.tile([1, QS], BF, tag="rs", name="rs")
                        nc.vector.tensor_copy(rs[:], pso[hh][DH : DH + 1])
                        rbc = pbig.tile([64, QS], F32, tag="big", name="rbc")
                        nc.tensor.matmul(
                            rbc[:],
                            ones_sb[:],
                            rs[:],
                            start=True,
                            stop=True,
                        )
                        rrec = nrm.tile([64, QS], F32, tag="rrec", name="rrec")
                        nc.vector.reciprocal_approx_fast(rrec[:], rbc[:])
                        nc.vector.tensor_tensor(
                            ot[p][64 * hh : 64 * hh + 64],
                            pso[hh][0:DH],
                            rrec[:],
                            ALU.mult,
                        )
                # out-projection of this strip (RS overlaps the next strip),
                # then drain leftover projection steps, then the post-RS
                # path of the previous strip (whose RS has finished by now)
                for st in outproj_steps(s, ot):
                    st()
                pump(len(pending))
                for st in post_pending:
                    st()
                post_pending = post_steps(s)
            for st in post_pending:
                st()

    nc.compile()
    return nc


_NC = None
_RUNNER = None


def _get_runner():
    """Build the compiled 8-core PJRT callable once and cache it."""
    global _NC, _RUNNER
    if _RUNNER is not None:
        return _RUNNER

    import jax
    import numpy as _np
    from jax.sharding import Mesh, PartitionSpec
    from jax.experimental.shard_map import shard_map
    from concourse.bass2jax import (
        _bass_exec_p,
        install_neuronx_cc_hook,
        partition_id_tensor,
    )

    _NC = build_nc()
    nc = _NC
    install_neuronx_cc_hook()

    partition_name = nc.partition_id_tensor.name if nc.partition_id_tensor else None
    in_names = []
    out_names = []
    out_avals = []
    zero_outs = []
    for alloc in nc.m.functions[0].allocations:
        if not isinstance(alloc, mybir.MemoryLocationSet):
            continue
        name = alloc.memorylocations[0].name
        if alloc.kind == "ExternalInput":
            if name != partition_name:
                in_names.append(name)
        elif alloc.kind == "ExternalOutput":
            shape = tuple(alloc.tensor_shape)
            dtype = mybir.dt.np(alloc.dtype)
            out_names.append(name)
            out_avals.append(jax.core.ShapedArray(shape, dtype))
            zero_outs.append(_np.zeros(shape, dtype))
    n_params = len(in_names)
    n_outs = len(out_avals)
    all_in_names = list(in_names) + list(out_names)
    if partition_name is not None:
        all_in_names.append(partition_name)

    def _body(*args):
        operands = list(args)
        if partition_name is not None:
            operands.append(partition_id_tensor())
        outs = _bass_exec_p.bind(
            *operands,
            out_avals=tuple(out_avals),
            in_names=tuple(all_in_names),
            out_names=tuple(out_names),
            lowering_input_output_aliases=(),
            sim_require_finite=True,
            sim_require_nnan=True,
            nc=nc,
        )
        return tuple(outs)

    devices = jax.devices()[:N_CORES]
    mesh = Mesh(np.asarray(devices), ("core",))
    in_specs = (PartitionSpec("core"),) * (n_params + n_outs)
    out_specs = (PartitionSpec("core"),) * n_outs
    sharded = jax.jit(
        shard_map(
            _body, mesh=mesh, in_specs=in_specs, out_specs=out_specs, check_rep=False
        ),
        keep_unused=True,
    )

    def run(in_maps):
        per_core = [[_np.asarray(m[name]) for name in in_names] for m in in_maps]
        concat_in = [
            _np.concatenate([per_core[c][i] for c in range(N_CORES)], axis=0)
            for i in range(n_params)
        ]
        concat_zeros = [
            _np.zeros((N_CORES * z.shape[0], *z.shape[1:]), z.dtype)
            for z in zero_outs
        ]
        out_arrs = sharded(*concat_in, *concat_zeros)
        return [
            {
                name: _np.asarray(out_arrs[i]).reshape(
                    N_CORES, *out_avals[i].shape
                )[c]
                for i, name in enumerate(out_names)
            }
            for c in range(N_CORES)
        ]

    _RUNNER = run
    return run


def make_in_maps(query, key, value, Wq, bq, Wk, bk, Wv, bv, Wo, bo):
    from ml_dtypes import bfloat16

    query = np.asarray(query, dtype=np.float32)
    key = np.asarray(key, dtype=np.float32)
    value = np.asarray(value, dtype=np.float32)
    Wq = np.asarray(Wq, dtype=np.float32)
    bq = np.asarray(bq, dtype=np.float32)
    Wk = np.asarray(Wk, dtype=np.float32)
    Wv = np.asarray(Wv, dtype=np.float32)
    bv = np.asarray(bv, dtype=np.float32)
    Wo = np.asarray(Wo, dtype=np.float32)
    bo = np.asarray(bo, dtype=np.float32)

    xqT = [np.ascontiguousarray(query[b].T).astype(bfloat16) for b in range(B)]
    xkT = [np.ascontiguousarray(key[b].T).astype(bfloat16) for b in range(B)]
    xvT = [np.ascontiguousarray(value[b].T).astype(bfloat16) for b in range(B)]

    # K bias is softmax-invariant (constant per q row) -> dropped.
    # V bias: softmax rows sum to 1, so it contributes bv @ Wo.T -> fold
    # into the output bias.
    bo_eff = bo + bv @ Wo.T
    bo_b = np.ascontiguousarray(
        np.broadcast_to(bo_eff, (128, HID))
    ).astype(np.float32)

    # upper-triangular (incl diagonal) mask for the diagonal 128x128 block
    tri = (np.arange(128)[None, :] >= np.arange(128)[:, None]).astype(bfloat16)

    in_maps = []
    for c in range(N_CORES):
        b = c // GROUP
        g = c % GROUP
        hsl = slice(HG * g, HG * g + HG)
        wq_g = np.ascontiguousarray(Wq[hsl].T).astype(bfloat16)  # [1024, 256]
        wk_g = np.ascontiguousarray(Wk[hsl].T).astype(bfloat16)
        wv_g = np.ascontiguousarray(Wv[hsl].T).astype(bfloat16)
        w2_g = np.ascontiguousarray(Wo[:, hsl].T).astype(bfloat16)  # [256, 1024]
        bq_g = np.ascontiguousarray(
            bq[hsl].reshape(2, 128).T
        ).astype(np.float32)
        in_maps.append(
            {
                "xq": xqT[b],
                "xk": xkT[b],
                "xv": xvT[b],
                "wq": wq_g,
                "wk": wk_g,
                "wv": wv_g,
                "w2": w2_g,
                "bqv": bq_g,
                "bob": bo_b,
                "trim": tri,
            }
        )
    return in_maps


def assemble_output(results):
    # core with group rank r holds rows [512s + 128r, +128) of its batch in
    # out_chunk[s]
    out = np.empty((B, S, HID), dtype=np.float32)
    for b in range(B):
        for r in range(GROUP):
            chunk = results[GROUP * b + r]["out_chunk"]
            for s in range(NQS):
                out[b, QS * s + 128 * r : QS * s + 128 * r + 128] = chunk[s]
    return out


def kernel(**inputs) -> np.ndarray:
    in_maps = make_in_maps(**inputs)
    run = _get_runner()
    results = run(in_maps)
    return assemble_output(results)


if __name__ == "__main__":
    import reference

    inputs = {k: np.asarray(v) for k, v in reference.setup_inputs().items()}
    got = kernel(**inputs)
    want = np.asarray(reference.reference(**inputs))
    err = np.linalg.norm(got - want) / np.linalg.norm(want)
    print("Relative error:", err)
